# revision 8
# baseline (speedup 1.0000x reference)
"""Contrastive loss (GRACE-style semi_loss pair) on 8 trn2 NeuronCores.

Math (reference):
    a = z1 / ||z1||_row ; b = z2 / ||z2||_row         (N=8192, D=512)
    refl    = exp(a @ a.T / tau) ; between = exp(a @ b.T / tau)
    l1_i = -log(between_ii / (refl.sum(1) + between.sum(1) - refl_ii))
    l2   = same with (z2, z1) swapped
    loss = mean(0.5 * (l1 + l2))

Identities:
  - between2 rowsums = COLUMN sums of exp(a@b.T/tau): one cross-core
    reduction of [8192] floats, no 4th matmul.
  - refl_ii = exp(1/tau) exactly; between_ii needs only dab_i = a_i . b_i.
  - l1_i + l2_i = beta_i + ln(denom2_i) with
    beta_i = ln(denom1_i) - 2 dab_i / tau.

Design (v2):
  - Per core inputs: z1T/z2T [512,8192] fp32 (shared, the only big reads),
    z1l/z2l row-major local slices (norms + dab), z1lT/z2lT (stationary),
    selp (per-core 8x64 selector for SPMD-positional alpha writes).
  - Norms: local sumsq on DVE + Newton rsqrt; 1/norm bf16 AllGathered
    (32KB) while zT streams; no full row-major z reads at all.
  - Matmuls in fp8e4 (x16-scaled operands) with DoubleRow perf mode:
    K=256 per instruction, 2 instrs per [128,512] product.
  - PSUM per m: one [128,1536] tile = aa|ab|bb. ACT does ONE fused
    exp+rowsum over aa|ab (denom1 needs only the sum) and exp over bb;
    bb rowsum on DVE. Column sums of exp(ab) accumulate on the PE via
    ones-matmuls, deferred one m-step so the PE never waits on ACT.
  - Tail: ONE AllReduce over [colsums+alpha(8192) | alpha-block(8192) |
    sum-beta(1)]: the AR itself sums partial colsums AND adds alpha_j
    (positioned at its global slot by a selector matmul) so AR output IS
    denom2; every core then computes the final scalar locally.
"""

import numpy as np
from contextlib import ExitStack

import concourse.bass as bass
import concourse.tile as tile
from concourse import bacc, mybir
from concourse.bass_utils import run_bass_kernel_spmd

N = 8192
D = 512
P = 128
NCORES = 8
LOCAL = N // NCORES            # 1024 rows per core
M_CH = LOCAL // P              # 8 local row blocks of 128
N_CH = N // 512                # 16 column chunks of 512
KC = D // P                    # 4 contraction chunks of 128
SUPW = 1024                    # DMA super-chunk width (2 chunks)
N_SUP = N // SUPW              # 8 supers
TAU = 0.4
EXPD = float(np.exp(1.0 / TAU))
Y0 = float(D) ** -0.5          # Newton rsqrt seed
FSC = 16.0                     # fp8 operand scale
ES = 1.0 / (FSC * FSC * TAU)   # exp scale on S' = 256*S

FP32 = mybir.dt.float32
BF16 = mybir.dt.bfloat16
FP8 = mybir.dt.float8e4
ALU = mybir.AluOpType
ACTF = mybir.ActivationFunctionType
DR = mybir.MatmulPerfMode.DoubleRow


def _build():
    nc = bacc.Bacc("TRN2", debug=False, num_devices=NCORES)
    z1T = nc.dram_tensor("z1T", [D, N], FP32, kind="ExternalInput").ap()
    z2T = nc.dram_tensor("z2T", [D, N], FP32, kind="ExternalInput").ap()
    z1l = nc.dram_tensor("z1l", [LOCAL, D], FP32, kind="ExternalInput").ap()
    z2l = nc.dram_tensor("z2l", [LOCAL, D], FP32, kind="ExternalInput").ap()
    z1lT = nc.dram_tensor("z1lT", [D, LOCAL], FP32, kind="ExternalInput").ap()
    z2lT = nc.dram_tensor("z2lT", [D, LOCAL], FP32, kind="ExternalInput").ap()
    selp = nc.dram_tensor("selp", [M_CH, 64], FP32, kind="ExternalInput").ap()
    loss = nc.dram_tensor("loss", [1, 1], FP32, kind="ExternalOutput").ap()

    with tile.TileContext(nc) as tc, ExitStack() as ctx:
        big = ctx.enter_context(tc.tile_pool(name="big", bufs=1))
        stg = ctx.enter_context(tc.tile_pool(name="stg", bufs=2))
        rowz = ctx.enter_context(tc.tile_pool(name="rowz", bufs=4))
        scr = ctx.enter_context(tc.tile_pool(name="scr", bufs=2))
        atp = ctx.enter_context(tc.tile_pool(name="atp", bufs=4))
        eabp = ctx.enter_context(tc.tile_pool(name="eabp", bufs=4))
        ebbp = ctx.enter_context(tc.tile_pool(name="ebbp", bufs=3))
        pmm = ctx.enter_context(tc.tile_pool(name="pmm", bufs=2, space="PSUM"))
        pbc = ctx.enter_context(tc.tile_pool(name="pbc", bufs=1, space="PSUM"))
        pcol = ctx.enter_context(tc.tile_pool(name="pcol", bufs=1, space="PSUM"))
        dram = ctx.enter_context(tc.tile_pool(name="dram", bufs=1, space="DRAM"))

        # ---- constants --------------------------------------------------
        ones_col = big.tile([P, 1], BF16, tag="ones_col", name="ones_col")
        nc.vector.memset(ones_col, 1.0)
        ones_f32 = big.tile([P, 1], FP32, tag="ones_f32", name="ones_f32")
        nc.vector.memset(ones_f32, 1.0)
        ones_row = big.tile([1, P], BF16, tag="ones_row", name="ones_row")
        nc.vector.memset(ones_row, 1.0)

        # ---- persistent tiles -------------------------------------------
        ATL1 = big.tile([P, KC, LOCAL], FP8, tag="ATL1", name="ATL1")
        ATL2 = big.tile([P, KC, LOCAL], FP8, tag="ATL2", name="ATL2")
        invnb1 = big.tile([P, LOCAL], BF16, tag="invnb1", name="invnb1")
        invnb2 = big.tile([P, LOCAL], BF16, tag="invnb2", name="invnb2")
        ivall = big.tile([1, 2 * N], BF16, tag="ivall", name="ivall")
        sel_sb = big.tile([M_CH, 64], FP32, tag="sel_sb", name="sel_sb")

        rsp1 = [
            big.tile([P, N_CH], FP32, tag=f"rsp1_{m}", name=f"rsp1_{m}")
            for m in range(M_CH)
        ]
        rsp2 = [
            big.tile([P, N_CH], FP32, tag=f"rsp2_{m}", name=f"rsp2_{m}")
            for m in range(M_CH)
        ]

        ss1 = big.tile([P, M_CH], FP32, tag="ss1", name="ss1")
        ss2 = big.tile([P, M_CH], FP32, tag="ss2", name="ss2")
        u_ab = big.tile([P, M_CH], FP32, tag="u_ab", name="u_ab")

        # collective buffers
        ag_in = dram.tile([1, 2 * LOCAL], BF16, tag="ag_in", name="ag_in")
        ag_out = dram.tile([1, 2 * N], BF16, tag="ag_out", name="ag_out")
        rs_in = dram.tile([1, 2 * N + 1], FP32, tag="rs_in", name="rs_in")
        rs_out = dram.tile(
            [1, 2 * N + 1], FP32, tag="rs_out", name="rs_out", addr_space="Shared"
        )

        GROUPS = [list(range(NCORES))]

        def sumsq(zt, acc_slice, nm, other=None):
            sq = scr.tile([P, D], BF16, tag="sq", name=f"sq_{nm}", bufs=2)
            nc.vector.scalar_tensor_tensor(
                out=sq, in0=zt, scalar=1.0,
                in1=other if other is not None else zt,
                op0=ALU.mult, op1=ALU.mult, accum_out=acc_slice,
            )

        def rsqrt_newton(ss, w, nm, iters=3):
            ssh = scr.tile([P, w], FP32, tag="rq_ssh", name=f"ssh_{nm}")
            nc.vector.tensor_scalar_mul(ssh, ss, 0.5)
            y = scr.tile([P, w], FP32, tag="rq_y", name=f"y_{nm}")
            nc.vector.tensor_scalar(
                out=y, in0=ssh, scalar1=-(Y0**3), scalar2=1.5 * Y0,
                op0=ALU.mult, op1=ALU.add,
            )
            t = scr.tile([P, w], FP32, tag="rq_t", name=f"t_{nm}")
            u = scr.tile([P, w], FP32, tag="rq_u", name=f"u_{nm}")
            for _ in range(iters - 1):
                nc.vector.tensor_mul(t, y, y)
                nc.vector.tensor_mul(t, t, ssh)
                nc.vector.tensor_mul(u, y, t)
                nc.vector.scalar_tensor_tensor(
                    out=y, in0=y, scalar=1.5, in1=u,
                    op0=ALU.mult, op1=ALU.subtract,
                )
            return y

        # ---- head DMAs (sync queue: local rows, stationary, supers) -----
        r1 = []
        r2 = []
        for t in range(M_CH):
            zt1 = rowz.tile([P, D], FP32, tag="r1", name=f"zl1_{t}")
            nc.sync.dma_start(out=zt1, in_=z1l[P * t : P * (t + 1), :])
            r1.append(zt1)
            zt2 = rowz.tile([P, D], FP32, tag="r2", name=f"zl2_{t}")
            nc.sync.dma_start(out=zt2, in_=z2l[P * t : P * (t + 1), :])
            r2.append(zt2)
        sl1 = big.tile([P, KC, LOCAL], FP32, tag="sl1", name="sl1")
        nc.sync.dma_start(
            out=sl1, in_=z1lT.rearrange("(k p) n -> p k n", p=P)
        )
        sl2 = big.tile([P, KC, LOCAL], FP32, tag="sl2", name="sl2")
        nc.sync.dma_start(
            out=sl2, in_=z2lT.rearrange("(k p) n -> p k n", p=P)
        )
        nc.scalar.dma_start(out=sel_sb, in_=selp)

        st1 = {}
        st2 = {}

        def prefetch(s):
            st1[s] = stg.tile([P, KC, SUPW], FP32, tag="st1", name=f"st1_{s}")
            nc.sync.dma_start(
                out=st1[s],
                in_=z1T.rearrange("(k p) n -> p k n", p=P)[
                    :, :, SUPW * s : SUPW * (s + 1)
                ],
            )
            st2[s] = stg.tile([P, KC, SUPW], FP32, tag="st2", name=f"st2_{s}")
            nc.sync.dma_start(
                out=st2[s],
                in_=z2T.rearrange("(k p) n -> p k n", p=P)[
                    :, :, SUPW * s : SUPW * (s + 1)
                ],
            )

        prefetch(0)
        prefetch(1)

        # ---- local norms -> AllGather (critical chain first) ------------
        # all three consumers of a row tile issue together so the rowz pool
        # (bufs=4) releases slots before later row DMAs need them
        for t in range(M_CH):
            sumsq(r1[t], ss1[:, t : t + 1], f"l1_{t}")
            sumsq(r2[t], ss2[:, t : t + 1], f"l2_{t}")
            sumsq(r1[t], u_ab[:, t : t + 1], f"u_{t}", other=r2[t])
        inv1 = rsqrt_newton(ss1, M_CH, "l1")
        inv2 = rsqrt_newton(ss2, M_CH, "l2")

        ivcl = scr.tile([P, 2 * M_CH], BF16, tag="ivcl", name="ivcl")
        nc.vector.tensor_copy(ivcl[:, 0:M_CH], inv1)
        nc.vector.tensor_copy(ivcl[:, M_CH : 2 * M_CH], inv2)
        # the whole AllGather chain rides the gpsimd software DGE: its
        # descriptors bypass the HWDGE queues, which at this point hold
        # megabytes of zT prefetch backlog that would delay the collective
        nc.gpsimd.dma_start(
            out=ag_in[:, 0:LOCAL].rearrange("o (t p) -> p (o t)", p=P),
            in_=ivcl[:, 0:M_CH],
        )
        nc.gpsimd.dma_start(
            out=ag_in[:, LOCAL : 2 * LOCAL].rearrange("o (t p) -> p (o t)", p=P),
            in_=ivcl[:, M_CH : 2 * M_CH],
        )
        nc.gpsimd.dma_start(
            out=invnb1, in_=ag_in[:, 0:LOCAL].to_broadcast([P, LOCAL])
        )
        nc.gpsimd.dma_start(
            out=invnb2, in_=ag_in[:, LOCAL : 2 * LOCAL].to_broadcast([P, LOCAL])
        )
        nc.gpsimd.collective_compute(
            "AllGather",
            ALU.bypass,
            replica_groups=GROUPS,
            ins=[ag_in.opt()],
            outs=[ag_out.opt()],
        )
        nc.gpsimd.dma_start(out=ivall, in_=ag_out)

        # dab + stationary fp8 operands (off the AG critical path)
        dab = big.tile([P, M_CH], FP32, tag="dab", name="dab")
        nc.vector.tensor_mul(dab, u_ab, inv1)
        nc.vector.tensor_mul(dab, dab, inv2)

        for k in range(KC):
            nc.vector.scalar_tensor_tensor(
                out=ATL1[:, k, :], in0=sl1[:, k, :], scalar=FSC, in1=invnb1,
                op0=ALU.mult, op1=ALU.mult,
            )
            nc.vector.scalar_tensor_tensor(
                out=ATL2[:, k, :], in0=sl2[:, k, :], scalar=FSC, in1=invnb2,
                op0=ALU.mult, op1=ALU.mult,
            )

        # ---- main loop --------------------------------------------------
        AT1 = {}
        AT2 = {}

        def prep(n):
            """Broadcast 1/norms for chunk n and scale zT slices to fp8."""
            s, h = n // 2, n % 2
            off = 512 * h
            AT1[n] = atp.tile([P, KC, 512], FP8, tag="AT1", name=f"AT1_{n}")
            AT2[n] = atp.tile([P, KC, 512], FP8, tag="AT2", name=f"AT2_{n}")
            # rows 512n..512(n+1) belong to core cblk = n//2, half h; the
            # AllGathered layout per core block is [inv1(1024) | inv2(1024)]
            cblk = n // 2
            base = 2 * LOCAL * cblk
            iv1 = ivall[0:1, base + 512 * h : base + 512 * h + 512]
            iv2 = ivall[0:1, base + LOCAL + 512 * h : base + LOCAL + 512 * h + 512]
            pb1 = pbc.tile([P, 512], FP32, tag="pb", name=f"pb1_{n}")
            nc.tensor.matmul(pb1, ones_row, iv1, start=True, stop=True)
            pbb1 = scr.tile([P, 512], BF16, tag="pbb1", name=f"pbb1_{n}")
            nc.vector.tensor_copy(pbb1, pb1)
            pb2 = pbc.tile([P, 512], FP32, tag="pb", name=f"pb2_{n}")
            nc.tensor.matmul(pb2, ones_row, iv2, start=True, stop=True)
            pbb2 = scr.tile([P, 512], BF16, tag="pbb2", name=f"pbb2_{n}")
            nc.vector.tensor_copy(pbb2, pb2)
            for k in range(KC):
                nc.vector.scalar_tensor_tensor(
                    out=AT1[n][:, k, :], in0=st1[s][:, k, off : off + 512],
                    scalar=FSC, in1=pbb1, op0=ALU.mult, op1=ALU.mult,
                )
            for k in range(KC):
                nc.vector.scalar_tensor_tensor(
                    out=AT2[n][:, k, :], in0=st2[s][:, k, off : off + 512],
                    scalar=FSC, in1=pbb2, op0=ALU.mult, op1=ALU.mult,
                )

        # column sums: bf16 accumulation on DVE per chunk, folded by a single
        # ones-matmul deferred into the NEXT chunk (PE never waits on ACT/DVE)
        colacc = {}
        pend_fold = []

        def flush_fold():
            if not pend_fold:
                return
            n = pend_fold.pop(0)
            colp = pcol.tile([1, 512], FP32, tag="col", name=f"colp_{n}")
            nc.tensor.matmul(colp, ones_col, colacc[n], start=True, stop=True)
            csb = scr.tile([1, 512], FP32, tag="csb", name=f"csb_{n}")
            nc.vector.tensor_copy(csb, colp)
            nc.scalar.dma_start(out=rs_in[:, 512 * n : 512 * (n + 1)], in_=csb)

        def main_chunk(n):
            for m in range(M_CH):
                mm = pmm.tile([P, 1536], FP32, tag="mm", name=f"mm_{n}_{m}")
                lhs1 = ATL1[:, :, P * m : P * (m + 1)]
                lhs2 = ATL2[:, :, P * m : P * (m + 1)]
                for half, (lo, hi) in enumerate(((0, 2), (2, 4))):
                    nc.tensor.matmul(
                        mm[:, 0:512], lhs1[:, lo:hi, :], AT1[n][:, lo:hi, :],
                        start=(half == 0), stop=(half == 1), perf_mode=DR,
                    )
                for half, (lo, hi) in enumerate(((0, 2), (2, 4))):
                    nc.tensor.matmul(
                        mm[:, 512:1024], lhs1[:, lo:hi, :], AT2[n][:, lo:hi, :],
                        start=(half == 0), stop=(half == 1), perf_mode=DR,
                    )
                for half, (lo, hi) in enumerate(((0, 2), (2, 4))):
                    nc.tensor.matmul(
                        mm[:, 1024:1536], lhs2[:, lo:hi, :], AT2[n][:, lo:hi, :],
                        start=(half == 0), stop=(half == 1), perf_mode=DR,
                    )
                if m == 1:
                    flush_fold()
                eab = eabp.tile([P, 1024], BF16, tag="eab", name=f"eab_{n}_{m}")
                nc.scalar.activation(
                    out=eab, in_=mm[:, 0:1024], func=ACTF.Exp, scale=ES,
                    accum_out=rsp1[m][:, n : n + 1],
                )
                ebb = ebbp.tile([P, 512], BF16, tag="ebb", name=f"ebb_{n}_{m}")
                nc.scalar.activation(
                    out=ebb, in_=mm[:, 1024:1536], func=ACTF.Exp, scale=ES,
                    accum_out=rsp2[m][:, n : n + 1],
                )
                if m == 0:
                    colacc[n] = scr.tile(
                        [P, 512], BF16, tag="colacc", name=f"colacc_{n}"
                    )
                    nc.vector.tensor_copy(colacc[n], eab[:, 512:1024])
                else:
                    nc.vector.tensor_add(
                        colacc[n], colacc[n], eab[:, 512:1024]
                    )
            pend_fold.append(n)

        # software pipeline: operand prep one chunk ahead, supers two ahead
        prep(0)
        prep(1)
        for n in range(N_CH):
            if n % 2 == 0 and n // 2 + 2 < N_SUP:
                prefetch(n // 2 + 2)
            if n + 2 < N_CH:
                prep(n + 2)
            main_chunk(n)
        while pend_fold:
            flush_fold()

        # ---- tail -------------------------------------------------------
        rs1 = big.tile([P, M_CH], FP32, tag="rs1", name="rs1")
        rs2 = big.tile([P, M_CH], FP32, tag="rs2", name="rs2")
        for m in range(M_CH):
            nc.vector.reduce_sum(
                out=rs1[:, m : m + 1], in_=rsp1[m], axis=mybir.AxisListType.X
            )
            nc.vector.reduce_sum(
                out=rs2[:, m : m + 1], in_=rsp2[m], axis=mybir.AxisListType.X
            )

        # beta = ln(denom1) - 2 dab / tau ; sum over local rows
        denom1 = scr.tile([P, M_CH], FP32, tag="denom1", name="denom1")
        nc.vector.tensor_scalar_add(denom1, rs1, -EXPD)
        nc.scalar.activation(out=denom1, in_=denom1, func=ACTF.Ln)
        combo = scr.tile([P, M_CH], FP32, tag="combo", name="combo")
        ppart = big.tile([P, 1], FP32, tag="ppart", name="ppart")
        nc.vector.scalar_tensor_tensor(
            out=combo, in0=dab, scalar=-2.0 / TAU, in1=denom1,
            op0=ALU.mult, op1=ALU.add, accum_out=ppart,
        )
        lps = pcol.tile([1, 512], FP32, tag="col", name="lps")
        nc.tensor.matmul(lps[0:1, 0:1], ones_f32, ppart, start=True, stop=True)
        lsb = big.tile([1, 1], FP32, tag="lsb", name="lsb")
        nc.vector.tensor_copy(lsb, lps[0:1, 0:1])
        nc.scalar.dma_start(out=rs_in[:, 2 * N : 2 * N + 1], in_=lsb)

        # alpha = rs2 - EXPD, positioned at global row slot via selector
        alpha = scr.tile([P, M_CH], FP32, tag="alpha", name="alpha")
        nc.vector.tensor_scalar_add(alpha, rs2, -EXPD)
        alr = dram.tile([1, LOCAL], FP32, tag="alr", name="alr")
        nc.scalar.dma_start(
            out=alr.rearrange("o (t p) -> p (o t)", p=P), in_=alpha
        )
        alT = big.tile([M_CH, P], FP32, tag="alT", name="alT")
        nc.scalar.dma_start(
            out=alT, in_=alr.rearrange("o (t p) -> t (o p)", p=P)
        )
        alf = pmm.tile([P, 1536], FP32, tag="mm", name="alf")
        nc.tensor.matmul(alf[0:64, 0:P], sel_sb, alT, start=True, stop=True)
        af_sb = big.tile([64, P], FP32, tag="af_sb", name="af_sb")
        nc.vector.tensor_copy(af_sb, alf[0:64, 0:P])
        nc.scalar.dma_start(
            out=rs_in[:, N : 2 * N].rearrange("o (t p) -> t (o p)", p=P),
            in_=af_sb,
        )

        nc.gpsimd.collective_compute(
            "AllReduce",
            ALU.add,
            replica_groups=GROUPS,
            ins=[rs_in.opt()],
            outs=[rs_out.opt()],
        )

        # final scalar: every core computes it (SPMD); core 0's is read
        cs_t = big.tile([P, 64], FP32, tag="cs_t", name="cs_t")
        nc.scalar.dma_start(
            out=cs_t, in_=rs_out[:, 0:N].rearrange("o (t p) -> p (o t)", p=P)
        )
        al_t = big.tile([P, 64], FP32, tag="al_t", name="al_t")
        nc.scalar.dma_start(
            out=al_t, in_=rs_out[:, N : 2 * N].rearrange("o (t p) -> p (o t)", p=P)
        )
        sb_t = big.tile([1, 1], FP32, tag="sb_t", name="sb_t")
        nc.scalar.dma_start(out=sb_t, in_=rs_out[:, 2 * N : 2 * N + 1])

        dn2 = big.tile([P, 64], FP32, tag="dn2", name="dn2")
        nc.vector.tensor_add(dn2, cs_t, al_t)
        nc.scalar.activation(out=dn2, in_=dn2, func=ACTF.Ln)
        lnp = big.tile([P, 1], FP32, tag="lnp", name="lnp")
        nc.vector.reduce_sum(out=lnp, in_=dn2, axis=mybir.AxisListType.X)
        tl2 = pcol.tile([1, 512], FP32, tag="col", name="tl2")
        nc.tensor.matmul(tl2[0:1, 0:1], ones_f32, lnp, start=True, stop=True)
        tot = big.tile([1, 1], FP32, tag="tot", name="tot")
        nc.vector.tensor_add(tot, tl2[0:1, 0:1], sb_t)
        nc.scalar.mul(tot, tot, 0.5 / N)
        nc.scalar.dma_start(out=loss, in_=tot)

    nc.compile()
    return nc


_NC_CACHE = None


def _get_nc():
    global _NC_CACHE
    if _NC_CACHE is None:
        _NC_CACHE = _build()
    return _NC_CACHE


def _in_maps(z1, z2):
    z1 = np.ascontiguousarray(np.asarray(z1), dtype=np.float32)
    z2 = np.ascontiguousarray(np.asarray(z2), dtype=np.float32)
    z1T = np.ascontiguousarray(z1.T)
    z2T = np.ascontiguousarray(z2.T)
    maps = []
    for c in range(NCORES):
        sl = slice(LOCAL * c, LOCAL * (c + 1))
        sel = np.zeros((M_CH, 64), dtype=np.float32)
        for i in range(M_CH):
            sel[i, M_CH * c + i] = 1.0
        maps.append(
            {
                "z1T": z1T,
                "z2T": z2T,
                "z1l": np.ascontiguousarray(z1[sl]),
                "z2l": np.ascontiguousarray(z2[sl]),
                "z1lT": np.ascontiguousarray(z1T[:, sl]),
                "z2lT": np.ascontiguousarray(z2T[:, sl]),
                "selp": sel,
            }
        )
    return maps


def kernel(z1, z2):
    nc = _get_nc()
    res = run_bass_kernel_spmd(nc, _in_maps(z1, z2), list(range(NCORES)))
    return np.asarray(res.results[0]["loss"], dtype=np.float32).reshape(())


def _install_ntff_hook_shim():
    """The agent image's antenv lacks axon_hooks; recreate the documented
    ctypes hook (same as trn_agent_boot.trn_boot._ntff_profile_via_ctypes)
    so run_bass_kernel_spmd(trace=True) can capture NTFF profiles."""
    import sys, types, ctypes, contextlib

    if "antenv.axon_hooks" in sys.modules:
        return
    so_path = "/opt/axon/libaxon_pjrt.so"
    lib = ctypes.CDLL(so_path)
    if not hasattr(lib, "axon_start_nrt_profile"):
        return
    lib.axon_start_nrt_profile.argtypes = [
        ctypes.POINTER(ctypes.c_int64),
        ctypes.c_size_t,
    ]
    lib.axon_start_nrt_profile.restype = ctypes.c_int64
    lib.axon_stop_nrt_profile.argtypes = [ctypes.c_char_p]
    lib.axon_stop_nrt_profile.restype = ctypes.c_int64

    @contextlib.contextmanager
    def _hook(output_dir, device_ids):
        import jax

        jax.devices()
        if device_ids:
            ids = (ctypes.c_int64 * len(device_ids))(*device_ids)
            rc = lib.axon_start_nrt_profile(ids, len(device_ids))
        else:
            rc = lib.axon_start_nrt_profile(None, 0)
        if rc != 0:
            raise RuntimeError(f"axon_start_nrt_profile rc={rc}")
        try:
            yield
        finally:
            n = lib.axon_stop_nrt_profile(str(output_dir).encode())
            if n < 0:
                raise RuntimeError(f"axon_stop_nrt_profile rc={n}")
            print(f"profile: {n} file(s) written to {output_dir}", file=sys.stderr)

    mod = types.ModuleType("antenv.axon_hooks")
    mod.get_axon_ntff_profile_hook = lambda: _hook
    mod.set_axon_ntff_profile_hook = lambda h: None
    sys.modules["antenv.axon_hooks"] = mod


def kernel_traced(z1, z2):
    """Same as kernel() but with NTFF profiling; returns (loss, exec_time_ns,
    trace_path)."""
    import concourse.bass_utils as bu

    _install_ntff_hook_shim()
    bu.upload_artifacts = lambda tmpdir: "local://" + tmpdir  # no egress
    nc = _get_nc()
    res = run_bass_kernel_spmd(
        nc, _in_maps(z1, z2), list(range(NCORES)), trace=True
    )
    out = np.asarray(res.results[0]["loss"], dtype=np.float32).reshape(())
    trace_path = (
        res.instructions_and_trace[1] if res.instructions_and_trace else None
    )
    return out, res.exec_time_ns, trace_path


# revision 12
# speedup vs baseline: 1.0539x; 1.0539x over previous
"""Contrastive loss (GRACE-style semi_loss pair) on 8 trn2 NeuronCores.

Math (reference):
    a = z1 / ||z1||_row ; b = z2 / ||z2||_row         (N=8192, D=512)
    refl    = exp(a @ a.T / tau) ; between = exp(a @ b.T / tau)
    l1_i = -log(between_ii / (refl.sum(1) + between.sum(1) - refl_ii))
    l2   = same with (z2, z1) swapped
    loss = mean(0.5 * (l1 + l2))

Identities:
  - between2 rowsums = COLUMN sums of exp(a@b.T/tau): one cross-core
    reduction of [8192] floats, no 4th matmul.
  - refl_ii = exp(1/tau) exactly; between_ii needs only dab_i = a_i . b_i.
  - l1_i + l2_i = beta_i + ln(denom2_i) with
    beta_i = ln(denom1_i) - 2 dab_i / tau.

Design (v2):
  - Per core inputs: z1T/z2T [512,8192] fp32 (shared, the only big reads),
    z1l/z2l row-major local slices (norms + dab), z1lT/z2lT (stationary),
    selp (per-core 8x64 selector for SPMD-positional alpha writes).
  - Norms: local sumsq on DVE + Newton rsqrt; 1/norm bf16 AllGathered
    (32KB) while zT streams; no full row-major z reads at all.
  - Matmuls in fp8e4 (x16-scaled operands) with DoubleRow perf mode:
    K=256 per instruction, 2 instrs per [128,512] product.
  - PSUM per m: one [128,1536] tile = aa|ab|bb. ACT does ONE fused
    exp+rowsum over aa|ab (denom1 needs only the sum) and exp over bb;
    bb rowsum on DVE. Column sums of exp(ab) accumulate on the PE via
    ones-matmuls, deferred one m-step so the PE never waits on ACT.
  - Tail: ONE AllReduce over [colsums+alpha(8192) | alpha-block(8192) |
    sum-beta(1)]: the AR itself sums partial colsums AND adds alpha_j
    (positioned at its global slot by a selector matmul) so AR output IS
    denom2; every core then computes the final scalar locally.
"""

import numpy as np
from contextlib import ExitStack

import concourse.bass as bass
import concourse.tile as tile
from concourse import bacc, mybir
from concourse.bass_utils import run_bass_kernel_spmd

N = 8192
D = 512
P = 128
NCORES = 8
LOCAL = N // NCORES            # 1024 rows per core
M_CH = LOCAL // P              # 8 local row blocks of 128
N_CH = N // 512                # 16 column chunks of 512
KC = D // P                    # 4 contraction chunks of 128
SUPW = 1024                    # DMA super-chunk width (2 chunks)
N_SUP = N // SUPW              # 8 supers
TAU = 0.4
EXPD = float(np.exp(1.0 / TAU))
Y0 = float(D) ** -0.5          # Newton rsqrt seed
FSC = 16.0                     # fp8 operand scale
ES = 1.0 / (FSC * FSC * TAU)   # exp scale on S' = 256*S

FP32 = mybir.dt.float32
BF16 = mybir.dt.bfloat16
FP8 = mybir.dt.float8e4
ALU = mybir.AluOpType
ACTF = mybir.ActivationFunctionType
DR = mybir.MatmulPerfMode.DoubleRow


def _build():
    nc = bacc.Bacc("TRN2", debug=False, num_devices=NCORES)
    z1T = nc.dram_tensor("z1T", [D, N], FP32, kind="ExternalInput").ap()
    z2T = nc.dram_tensor("z2T", [D, N], FP32, kind="ExternalInput").ap()
    z1l = nc.dram_tensor("z1l", [LOCAL, D], FP32, kind="ExternalInput").ap()
    z2l = nc.dram_tensor("z2l", [LOCAL, D], FP32, kind="ExternalInput").ap()
    z1lT = nc.dram_tensor("z1lT", [D, LOCAL], FP32, kind="ExternalInput").ap()
    z2lT = nc.dram_tensor("z2lT", [D, LOCAL], FP32, kind="ExternalInput").ap()
    selp = nc.dram_tensor("selp", [M_CH, 64], FP32, kind="ExternalInput").ap()
    loss = nc.dram_tensor("loss", [1, 1], FP32, kind="ExternalOutput").ap()

    with tile.TileContext(nc) as tc, ExitStack() as ctx:
        big = ctx.enter_context(tc.tile_pool(name="big", bufs=1))
        stg = ctx.enter_context(tc.tile_pool(name="stg", bufs=2))
        rowz = ctx.enter_context(tc.tile_pool(name="rowz", bufs=4))
        scr = ctx.enter_context(tc.tile_pool(name="scr", bufs=2))
        atp = ctx.enter_context(tc.tile_pool(name="atp", bufs=4))
        eabp = ctx.enter_context(tc.tile_pool(name="eabp", bufs=8))
        ebbp = ctx.enter_context(tc.tile_pool(name="ebbp", bufs=3))
        pmm = ctx.enter_context(tc.tile_pool(name="pmm", bufs=2, space="PSUM"))
        pbc = ctx.enter_context(tc.tile_pool(name="pbc", bufs=1, space="PSUM"))
        pcol = ctx.enter_context(tc.tile_pool(name="pcol", bufs=1, space="PSUM"))
        dram = ctx.enter_context(tc.tile_pool(name="dram", bufs=1, space="DRAM"))

        # ---- constants --------------------------------------------------
        ones_col = big.tile([P, 1], BF16, tag="ones_col", name="ones_col")
        nc.vector.memset(ones_col, 1.0)
        ones_f32 = big.tile([P, 1], FP32, tag="ones_f32", name="ones_f32")
        nc.vector.memset(ones_f32, 1.0)
        ones_row = big.tile([1, P], BF16, tag="ones_row", name="ones_row")
        nc.vector.memset(ones_row, 1.0)

        # ---- persistent tiles -------------------------------------------
        ATL1 = big.tile([P, KC, LOCAL], FP8, tag="ATL1", name="ATL1")
        ATL2 = big.tile([P, KC, LOCAL], FP8, tag="ATL2", name="ATL2")
        invnb1 = big.tile([P, LOCAL], BF16, tag="invnb1", name="invnb1")
        invnb2 = big.tile([P, LOCAL], BF16, tag="invnb2", name="invnb2")
        ivall = big.tile([1, 2 * N], BF16, tag="ivall", name="ivall")
        sel_sb = big.tile([M_CH, 64], FP32, tag="sel_sb", name="sel_sb")

        rsp1 = [
            big.tile([P, N_CH], FP32, tag=f"rsp1_{m}", name=f"rsp1_{m}")
            for m in range(M_CH)
        ]
        rsp2 = [
            big.tile([P, N_CH], FP32, tag=f"rsp2_{m}", name=f"rsp2_{m}")
            for m in range(M_CH)
        ]

        ss1 = big.tile([P, M_CH], FP32, tag="ss1", name="ss1")
        ss2 = big.tile([P, M_CH], FP32, tag="ss2", name="ss2")
        u_ab = big.tile([P, M_CH], FP32, tag="u_ab", name="u_ab")

        # collective buffers
        ag_in = dram.tile([1, 2 * LOCAL], BF16, tag="ag_in", name="ag_in")
        ag_out = dram.tile([1, 2 * N], BF16, tag="ag_out", name="ag_out")
        rs_in = dram.tile([1, 2 * N + 1], FP32, tag="rs_in", name="rs_in")
        rs_out = dram.tile(
            [1, 2 * N + 1], FP32, tag="rs_out", name="rs_out", addr_space="Shared"
        )

        GROUPS = [list(range(NCORES))]

        def sumsq(zt, acc_slice, nm, other=None):
            sq = scr.tile([P, D], BF16, tag="sq", name=f"sq_{nm}", bufs=2)
            nc.vector.scalar_tensor_tensor(
                out=sq, in0=zt, scalar=1.0,
                in1=other if other is not None else zt,
                op0=ALU.mult, op1=ALU.mult, accum_out=acc_slice,
            )

        def rsqrt_newton(ss, w, nm, iters=3):
            ssh = scr.tile([P, w], FP32, tag="rq_ssh", name=f"ssh_{nm}")
            nc.vector.tensor_scalar_mul(ssh, ss, 0.5)
            y = scr.tile([P, w], FP32, tag="rq_y", name=f"y_{nm}")
            nc.vector.tensor_scalar(
                out=y, in0=ssh, scalar1=-(Y0**3), scalar2=1.5 * Y0,
                op0=ALU.mult, op1=ALU.add,
            )
            t = scr.tile([P, w], FP32, tag="rq_t", name=f"t_{nm}")
            u = scr.tile([P, w], FP32, tag="rq_u", name=f"u_{nm}")
            for _ in range(iters - 1):
                nc.vector.tensor_mul(t, y, y)
                nc.vector.tensor_mul(t, t, ssh)
                nc.vector.tensor_mul(u, y, t)
                nc.vector.scalar_tensor_tensor(
                    out=y, in0=y, scalar=1.5, in1=u,
                    op0=ALU.mult, op1=ALU.subtract,
                )
            return y

        # ---- head DMAs (sync queue: local rows, stationary, supers) -----
        r1 = []
        r2 = []
        for t in range(M_CH):
            zt1 = rowz.tile([P, D], FP32, tag="r1", name=f"zl1_{t}")
            nc.sync.dma_start(out=zt1, in_=z1l[P * t : P * (t + 1), :])
            r1.append(zt1)
            zt2 = rowz.tile([P, D], FP32, tag="r2", name=f"zl2_{t}")
            nc.sync.dma_start(out=zt2, in_=z2l[P * t : P * (t + 1), :])
            r2.append(zt2)
        nc.scalar.dma_start(out=sel_sb, in_=selp)

        # ---- local norms -> AllGather (critical chain first) ------------
        # all three consumers of a row tile issue together so the rowz pool
        # (bufs=4) releases slots before later row DMAs need them
        for t in range(M_CH):
            sumsq(r1[t], ss1[:, t : t + 1], f"l1_{t}")
            sumsq(r2[t], ss2[:, t : t + 1], f"l2_{t}")
            sumsq(r1[t], u_ab[:, t : t + 1], f"u_{t}", other=r2[t])
        inv1 = rsqrt_newton(ss1, M_CH, "l1")
        inv2 = rsqrt_newton(ss2, M_CH, "l2")

        ivcl = scr.tile([P, 2 * M_CH], BF16, tag="ivcl", name="ivcl")
        nc.vector.tensor_copy(ivcl[:, 0:M_CH], inv1)
        nc.vector.tensor_copy(ivcl[:, M_CH : 2 * M_CH], inv2)
        # ag_in writes ride the sync HWDGE ring BEFORE the sl/super loads
        # are triggered: descriptor lines interleave round-robin over the
        # shared DMA queues, so trigger order decides completion order —
        # this keeps the collective input from queueing behind megabytes
        # of zT prefetch traffic
        nc.sync.dma_start(
            out=ag_in[:, 0:LOCAL].rearrange("o (t p) -> p (o t)", p=P),
            in_=ivcl[:, 0:M_CH],
        )
        nc.sync.dma_start(
            out=ag_in[:, LOCAL : 2 * LOCAL].rearrange("o (t p) -> p (o t)", p=P),
            in_=ivcl[:, M_CH : 2 * M_CH],
        )
        nc.gpsimd.dma_start(
            out=invnb1, in_=ag_in[:, 0:LOCAL].to_broadcast([P, LOCAL])
        )
        nc.gpsimd.dma_start(
            out=invnb2, in_=ag_in[:, LOCAL : 2 * LOCAL].to_broadcast([P, LOCAL])
        )
        nc.gpsimd.collective_compute(
            "AllGather",
            ALU.bypass,
            replica_groups=GROUPS,
            ins=[ag_in.opt()],
            outs=[ag_out.opt()],
        )
        nc.gpsimd.dma_start(out=ivall, in_=ag_out)

        # big streaming loads trigger only after the ag_in writes
        sl1 = big.tile([P, KC, LOCAL], FP32, tag="sl1", name="sl1")
        nc.sync.dma_start(
            out=sl1, in_=z1lT.rearrange("(k p) n -> p k n", p=P)
        )
        sl2 = big.tile([P, KC, LOCAL], FP32, tag="sl2", name="sl2")
        nc.sync.dma_start(
            out=sl2, in_=z2lT.rearrange("(k p) n -> p k n", p=P)
        )

        st1 = {}
        st2 = {}

        def prefetch(s):
            st1[s] = stg.tile([P, KC, SUPW], FP32, tag="st1", name=f"st1_{s}")
            nc.sync.dma_start(
                out=st1[s],
                in_=z1T.rearrange("(k p) n -> p k n", p=P)[
                    :, :, SUPW * s : SUPW * (s + 1)
                ],
            )
            st2[s] = stg.tile([P, KC, SUPW], FP32, tag="st2", name=f"st2_{s}")
            nc.sync.dma_start(
                out=st2[s],
                in_=z2T.rearrange("(k p) n -> p k n", p=P)[
                    :, :, SUPW * s : SUPW * (s + 1)
                ],
            )

        prefetch(0)
        prefetch(1)

        # dab + stationary fp8 operands (off the AG critical path)
        dab = big.tile([P, M_CH], FP32, tag="dab", name="dab")
        nc.vector.tensor_mul(dab, u_ab, inv1)
        nc.vector.tensor_mul(dab, dab, inv2)

        for k in range(KC):
            nc.vector.scalar_tensor_tensor(
                out=ATL1[:, k, :], in0=sl1[:, k, :], scalar=FSC, in1=invnb1,
                op0=ALU.mult, op1=ALU.mult,
            )
            nc.vector.scalar_tensor_tensor(
                out=ATL2[:, k, :], in0=sl2[:, k, :], scalar=FSC, in1=invnb2,
                op0=ALU.mult, op1=ALU.mult,
            )

        # ---- main loop --------------------------------------------------
        AT1 = {}
        AT2 = {}

        def prep(n):
            """Broadcast 1/norms for chunk n and scale zT slices to fp8."""
            s, h = n // 2, n % 2
            off = 512 * h
            AT1[n] = atp.tile([P, KC, 512], FP8, tag="AT1", name=f"AT1_{n}")
            AT2[n] = atp.tile([P, KC, 512], FP8, tag="AT2", name=f"AT2_{n}")
            # rows 512n..512(n+1) belong to core cblk = n//2, half h; the
            # AllGathered layout per core block is [inv1(1024) | inv2(1024)]
            cblk = n // 2
            base = 2 * LOCAL * cblk
            iv1 = ivall[0:1, base + 512 * h : base + 512 * h + 512]
            iv2 = ivall[0:1, base + LOCAL + 512 * h : base + LOCAL + 512 * h + 512]
            pb1 = pbc.tile([P, 512], FP32, tag="pb", name=f"pb1_{n}")
            nc.tensor.matmul(pb1, ones_row, iv1, start=True, stop=True)
            pbb1 = scr.tile([P, 512], BF16, tag="pbb1", name=f"pbb1_{n}")
            nc.vector.tensor_copy(pbb1, pb1)
            pb2 = pbc.tile([P, 512], FP32, tag="pb", name=f"pb2_{n}")
            nc.tensor.matmul(pb2, ones_row, iv2, start=True, stop=True)
            pbb2 = scr.tile([P, 512], BF16, tag="pbb2", name=f"pbb2_{n}")
            nc.vector.tensor_copy(pbb2, pb2)
            for k in range(KC):
                nc.vector.scalar_tensor_tensor(
                    out=AT1[n][:, k, :], in0=st1[s][:, k, off : off + 512],
                    scalar=FSC, in1=pbb1, op0=ALU.mult, op1=ALU.mult,
                )
            for k in range(KC):
                nc.vector.scalar_tensor_tensor(
                    out=AT2[n][:, k, :], in0=st2[s][:, k, off : off + 512],
                    scalar=FSC, in1=pbb2, op0=ALU.mult, op1=ALU.mult,
                )

        # column sums: bf16 accumulation on DVE per chunk, folded by a single
        # ones-matmul deferred into the NEXT chunk (PE never waits on ACT/DVE)
        colacc = {}
        pend_fold = []

        def flush_fold():
            if not pend_fold:
                return
            n = pend_fold.pop(0)
            colp = pcol.tile([1, 512], FP32, tag="col", name=f"colp_{n}")
            nc.tensor.matmul(colp, ones_col, colacc[n], start=True, stop=True)
            csb = scr.tile([1, 512], FP32, tag="csb", name=f"csb_{n}")
            nc.vector.tensor_copy(csb, colp)
            nc.sync.dma_start(out=rs_in[:, 512 * n : 512 * (n + 1)], in_=csb)

        def main_chunk(n):
            for m in range(M_CH):
                mm = pmm.tile([P, 1536], FP32, tag="mm", name=f"mm_{n}_{m}")
                lhs1 = ATL1[:, :, P * m : P * (m + 1)]
                lhs2 = ATL2[:, :, P * m : P * (m + 1)]
                for half, (lo, hi) in enumerate(((0, 2), (2, 4))):
                    nc.tensor.matmul(
                        mm[:, 0:512], lhs1[:, lo:hi, :], AT1[n][:, lo:hi, :],
                        start=(half == 0), stop=(half == 1), perf_mode=DR,
                    )
                for half, (lo, hi) in enumerate(((0, 2), (2, 4))):
                    nc.tensor.matmul(
                        mm[:, 512:1024], lhs1[:, lo:hi, :], AT2[n][:, lo:hi, :],
                        start=(half == 0), stop=(half == 1), perf_mode=DR,
                    )
                for half, (lo, hi) in enumerate(((0, 2), (2, 4))):
                    nc.tensor.matmul(
                        mm[:, 1024:1536], lhs2[:, lo:hi, :], AT2[n][:, lo:hi, :],
                        start=(half == 0), stop=(half == 1), perf_mode=DR,
                    )
                if m == 1:
                    flush_fold()
                eab = eabp.tile([P, 1024], BF16, tag="eab", name=f"eab_{n}_{m}")
                nc.scalar.activation(
                    out=eab, in_=mm[:, 0:1024], func=ACTF.Exp, scale=ES,
                    accum_out=rsp1[m][:, n : n + 1],
                )
                ebb = ebbp.tile([P, 512], BF16, tag="ebb", name=f"ebb_{n}_{m}")
                nc.scalar.activation(
                    out=ebb, in_=mm[:, 1024:1536], func=ACTF.Exp, scale=ES,
                    accum_out=rsp2[m][:, n : n + 1],
                )
                if m == 0:
                    colacc[n] = scr.tile(
                        [P, 512], BF16, tag="colacc", name=f"colacc_{n}"
                    )
                    nc.vector.tensor_copy(colacc[n], eab[:, 512:1024])
                else:
                    nc.vector.tensor_add(
                        colacc[n], colacc[n], eab[:, 512:1024]
                    )
            pend_fold.append(n)

        # software pipeline: operand prep one chunk ahead, supers two ahead.
        # prep(n+2) is issued AFTER main(n) so chunk-n colaccs precede the
        # next prep's scale ops in the DVE queue — otherwise exp stalls on
        # eab slots while the DVE chews through scales first.
        prep(0)
        prep(1)
        for n in range(N_CH):
            if n % 2 == 0 and n // 2 + 2 < N_SUP:
                prefetch(n // 2 + 2)
            main_chunk(n)
            if n + 2 < N_CH:
                prep(n + 2)
        while pend_fold:
            flush_fold()

        # ---- tail -------------------------------------------------------
        rs1 = big.tile([P, M_CH], FP32, tag="rs1", name="rs1")
        rs2 = big.tile([P, M_CH], FP32, tag="rs2", name="rs2")
        for m in range(M_CH):
            nc.vector.reduce_sum(
                out=rs1[:, m : m + 1], in_=rsp1[m], axis=mybir.AxisListType.X
            )
            nc.vector.reduce_sum(
                out=rs2[:, m : m + 1], in_=rsp2[m], axis=mybir.AxisListType.X
            )

        # beta = ln(denom1) - 2 dab / tau ; sum over local rows
        denom1 = scr.tile([P, M_CH], FP32, tag="denom1", name="denom1")
        nc.vector.tensor_scalar_add(denom1, rs1, -EXPD)
        nc.scalar.activation(out=denom1, in_=denom1, func=ACTF.Ln)
        combo = scr.tile([P, M_CH], FP32, tag="combo", name="combo")
        ppart = big.tile([P, 1], FP32, tag="ppart", name="ppart")
        nc.vector.scalar_tensor_tensor(
            out=combo, in0=dab, scalar=-2.0 / TAU, in1=denom1,
            op0=ALU.mult, op1=ALU.add, accum_out=ppart,
        )
        lps = pcol.tile([1, 512], FP32, tag="col", name="lps")
        nc.tensor.matmul(lps[0:1, 0:1], ones_f32, ppart, start=True, stop=True)
        lsb = big.tile([1, 1], FP32, tag="lsb", name="lsb")
        nc.vector.tensor_copy(lsb, lps[0:1, 0:1])
        nc.scalar.dma_start(out=rs_in[:, 2 * N : 2 * N + 1], in_=lsb)

        # alpha = rs2 - EXPD, positioned at global row slot via selector
        alpha = scr.tile([P, M_CH], FP32, tag="alpha", name="alpha")
        nc.vector.tensor_scalar_add(alpha, rs2, -EXPD)
        alr = dram.tile([1, LOCAL], FP32, tag="alr", name="alr")
        nc.scalar.dma_start(
            out=alr.rearrange("o (t p) -> p (o t)", p=P), in_=alpha
        )
        alT = big.tile([M_CH, P], FP32, tag="alT", name="alT")
        nc.scalar.dma_start(
            out=alT, in_=alr.rearrange("o (t p) -> t (o p)", p=P)
        )
        alf = pmm.tile([P, 1536], FP32, tag="mm", name="alf")
        nc.tensor.matmul(alf[0:64, 0:P], sel_sb, alT, start=True, stop=True)
        af_sb = big.tile([64, P], FP32, tag="af_sb", name="af_sb")
        nc.vector.tensor_copy(af_sb, alf[0:64, 0:P])
        nc.scalar.dma_start(
            out=rs_in[:, N : 2 * N].rearrange("o (t p) -> t (o p)", p=P),
            in_=af_sb,
        )

        nc.gpsimd.collective_compute(
            "AllReduce",
            ALU.add,
            replica_groups=GROUPS,
            ins=[rs_in.opt()],
            outs=[rs_out.opt()],
        )

        # final scalar: every core computes it (SPMD); core 0's is read
        cs_t = big.tile([P, 64], FP32, tag="cs_t", name="cs_t")
        nc.scalar.dma_start(
            out=cs_t, in_=rs_out[:, 0:N].rearrange("o (t p) -> p (o t)", p=P)
        )
        al_t = big.tile([P, 64], FP32, tag="al_t", name="al_t")
        nc.scalar.dma_start(
            out=al_t, in_=rs_out[:, N : 2 * N].rearrange("o (t p) -> p (o t)", p=P)
        )
        sb_t = big.tile([1, 1], FP32, tag="sb_t", name="sb_t")
        nc.scalar.dma_start(out=sb_t, in_=rs_out[:, 2 * N : 2 * N + 1])

        dn2 = big.tile([P, 64], FP32, tag="dn2", name="dn2")
        nc.vector.tensor_add(dn2, cs_t, al_t)
        nc.scalar.activation(out=dn2, in_=dn2, func=ACTF.Ln)
        lnp = big.tile([P, 1], FP32, tag="lnp", name="lnp")
        nc.vector.reduce_sum(out=lnp, in_=dn2, axis=mybir.AxisListType.X)
        tl2 = pcol.tile([1, 512], FP32, tag="col", name="tl2")
        nc.tensor.matmul(tl2[0:1, 0:1], ones_f32, lnp, start=True, stop=True)
        tot = big.tile([1, 1], FP32, tag="tot", name="tot")
        nc.vector.tensor_add(tot, tl2[0:1, 0:1], sb_t)
        nc.scalar.mul(tot, tot, 0.5 / N)
        nc.scalar.dma_start(out=loss, in_=tot)

    nc.compile()
    return nc


_NC_CACHE = None


def _get_nc():
    global _NC_CACHE
    if _NC_CACHE is None:
        _NC_CACHE = _build()
    return _NC_CACHE


def _in_maps(z1, z2):
    z1 = np.ascontiguousarray(np.asarray(z1), dtype=np.float32)
    z2 = np.ascontiguousarray(np.asarray(z2), dtype=np.float32)
    z1T = np.ascontiguousarray(z1.T)
    z2T = np.ascontiguousarray(z2.T)
    maps = []
    for c in range(NCORES):
        sl = slice(LOCAL * c, LOCAL * (c + 1))
        sel = np.zeros((M_CH, 64), dtype=np.float32)
        for i in range(M_CH):
            sel[i, M_CH * c + i] = 1.0
        maps.append(
            {
                "z1T": z1T,
                "z2T": z2T,
                "z1l": np.ascontiguousarray(z1[sl]),
                "z2l": np.ascontiguousarray(z2[sl]),
                "z1lT": np.ascontiguousarray(z1T[:, sl]),
                "z2lT": np.ascontiguousarray(z2T[:, sl]),
                "selp": sel,
            }
        )
    return maps


def kernel(z1, z2):
    nc = _get_nc()
    res = run_bass_kernel_spmd(nc, _in_maps(z1, z2), list(range(NCORES)))
    return np.asarray(res.results[0]["loss"], dtype=np.float32).reshape(())


def _install_ntff_hook_shim():
    """The agent image's antenv lacks axon_hooks; recreate the documented
    ctypes hook (same as trn_agent_boot.trn_boot._ntff_profile_via_ctypes)
    so run_bass_kernel_spmd(trace=True) can capture NTFF profiles."""
    import sys, types, ctypes, contextlib

    if "antenv.axon_hooks" in sys.modules:
        return
    so_path = "/opt/axon/libaxon_pjrt.so"
    lib = ctypes.CDLL(so_path)
    if not hasattr(lib, "axon_start_nrt_profile"):
        return
    lib.axon_start_nrt_profile.argtypes = [
        ctypes.POINTER(ctypes.c_int64),
        ctypes.c_size_t,
    ]
    lib.axon_start_nrt_profile.restype = ctypes.c_int64
    lib.axon_stop_nrt_profile.argtypes = [ctypes.c_char_p]
    lib.axon_stop_nrt_profile.restype = ctypes.c_int64

    @contextlib.contextmanager
    def _hook(output_dir, device_ids):
        import jax

        jax.devices()
        if device_ids:
            ids = (ctypes.c_int64 * len(device_ids))(*device_ids)
            rc = lib.axon_start_nrt_profile(ids, len(device_ids))
        else:
            rc = lib.axon_start_nrt_profile(None, 0)
        if rc != 0:
            raise RuntimeError(f"axon_start_nrt_profile rc={rc}")
        try:
            yield
        finally:
            n = lib.axon_stop_nrt_profile(str(output_dir).encode())
            if n < 0:
                raise RuntimeError(f"axon_stop_nrt_profile rc={n}")
            print(f"profile: {n} file(s) written to {output_dir}", file=sys.stderr)

    mod = types.ModuleType("antenv.axon_hooks")
    mod.get_axon_ntff_profile_hook = lambda: _hook
    mod.set_axon_ntff_profile_hook = lambda h: None
    sys.modules["antenv.axon_hooks"] = mod


def kernel_traced(z1, z2):
    """Same as kernel() but with NTFF profiling; returns (loss, exec_time_ns,
    trace_path)."""
    import concourse.bass_utils as bu

    _install_ntff_hook_shim()
    bu.upload_artifacts = lambda tmpdir: "local://" + tmpdir  # no egress
    nc = _get_nc()
    res = run_bass_kernel_spmd(
        nc, _in_maps(z1, z2), list(range(NCORES)), trace=True
    )
    out = np.asarray(res.results[0]["loss"], dtype=np.float32).reshape(())
    trace_path = (
        res.instructions_and_trace[1] if res.instructions_and_trace else None
    )
    return out, res.exec_time_ns, trace_path


# revision 15
# speedup vs baseline: 1.2118x; 1.1499x over previous
"""Contrastive loss (GRACE-style semi_loss pair) on 8 trn2 NeuronCores.

Math (reference):
    a = z1 / ||z1||_row ; b = z2 / ||z2||_row         (N=8192, D=512)
    refl    = exp(a @ a.T / tau) ; between = exp(a @ b.T / tau)
    l1_i = -log(between_ii / (refl.sum(1) + between.sum(1) - refl_ii))
    l2   = same with (z2, z1) swapped
    loss = mean(0.5 * (l1 + l2))

Identities:
  - between2 rowsums = COLUMN sums of exp(a@b.T/tau): one cross-core
    reduction of [8192] floats, no 4th matmul.
  - refl_ii = exp(1/tau) exactly; between_ii needs only dab_i = a_i . b_i.
  - l1_i + l2_i = beta_i + ln(denom2_i) with
    beta_i = ln(denom1_i) - 2 dab_i / tau.

Design (v2):
  - Per core inputs: z1T/z2T [512,8192] fp32 (shared, the only big reads),
    z1l/z2l row-major local slices (norms + dab), z1lT/z2lT (stationary),
    selp (per-core 8x64 selector for SPMD-positional alpha writes).
  - Norms: local sumsq on DVE + Newton rsqrt; 1/norm bf16 AllGathered
    (32KB) while zT streams; no full row-major z reads at all.
  - Matmuls in fp8e4 (x16-scaled operands) with DoubleRow perf mode:
    K=256 per instruction, 2 instrs per [128,512] product.
  - PSUM per m: one [128,1536] tile = aa|ab|bb. ACT does ONE fused
    exp+rowsum over aa|ab (denom1 needs only the sum) and exp over bb;
    bb rowsum on DVE. Column sums of exp(ab) accumulate on the PE via
    ones-matmuls, deferred one m-step so the PE never waits on ACT.
  - Tail: ONE AllReduce over [colsums+alpha(8192) | alpha-block(8192) |
    sum-beta(1)]: the AR itself sums partial colsums AND adds alpha_j
    (positioned at its global slot by a selector matmul) so AR output IS
    denom2; every core then computes the final scalar locally.
"""

import numpy as np
from contextlib import ExitStack

import concourse.bass as bass
import concourse.tile as tile
from concourse import bacc, mybir
from concourse.bass_utils import run_bass_kernel_spmd

N = 8192
D = 512
P = 128
NCORES = 8
LOCAL = N // NCORES            # 1024 rows per core
M_CH = LOCAL // P              # 8 local row blocks of 128
N_CH = N // 512                # 16 column chunks of 512
KC = D // P                    # 4 contraction chunks of 128
SUPW = 1024                    # DMA super-chunk width (2 chunks)
N_SUP = N // SUPW              # 8 supers
TAU = 0.4
EXPD = float(np.exp(1.0 / TAU))
Y0 = float(D) ** -0.5          # Newton rsqrt seed
FSC = 16.0                     # fp8 operand scale
ES = 1.0 / (FSC * FSC * TAU)   # exp scale on S' = 256*S

FP32 = mybir.dt.float32
BF16 = mybir.dt.bfloat16
FP8 = mybir.dt.float8e4
ALU = mybir.AluOpType
ACTF = mybir.ActivationFunctionType
DR = mybir.MatmulPerfMode.DoubleRow


def _build():
    nc = bacc.Bacc("TRN2", debug=False, num_devices=NCORES)
    z1T = nc.dram_tensor("z1T", [D, N], FP32, kind="ExternalInput").ap()
    z2T = nc.dram_tensor("z2T", [D, N], FP32, kind="ExternalInput").ap()
    z1l = nc.dram_tensor("z1l", [LOCAL, D], FP32, kind="ExternalInput").ap()
    z2l = nc.dram_tensor("z2l", [LOCAL, D], FP32, kind="ExternalInput").ap()
    z1lT = nc.dram_tensor("z1lT", [D, LOCAL], FP32, kind="ExternalInput").ap()
    z2lT = nc.dram_tensor("z2lT", [D, LOCAL], FP32, kind="ExternalInput").ap()
    selp = nc.dram_tensor("selp", [M_CH, 64], FP32, kind="ExternalInput").ap()
    loss = nc.dram_tensor("loss", [1, 1], FP32, kind="ExternalOutput").ap()

    with tile.TileContext(nc) as tc, ExitStack() as ctx:
        big = ctx.enter_context(tc.tile_pool(name="big", bufs=1))
        stg = ctx.enter_context(tc.tile_pool(name="stg", bufs=2))
        rowz = ctx.enter_context(tc.tile_pool(name="rowz", bufs=4))
        scr = ctx.enter_context(tc.tile_pool(name="scr", bufs=2))
        atp = ctx.enter_context(tc.tile_pool(name="atp", bufs=4))
        eabp = ctx.enter_context(tc.tile_pool(name="eabp", bufs=6))
        ebbp = ctx.enter_context(tc.tile_pool(name="ebbp", bufs=4))
        pmm = ctx.enter_context(tc.tile_pool(name="pmm", bufs=2, space="PSUM"))
        pbc = ctx.enter_context(tc.tile_pool(name="pbc", bufs=1, space="PSUM"))
        pcol = ctx.enter_context(tc.tile_pool(name="pcol", bufs=1, space="PSUM"))
        dram = ctx.enter_context(tc.tile_pool(name="dram", bufs=1, space="DRAM"))

        # ---- constants --------------------------------------------------
        ones_col = big.tile([P, 1], BF16, tag="ones_col", name="ones_col")
        nc.vector.memset(ones_col, 1.0)
        ones_f32 = big.tile([P, 1], FP32, tag="ones_f32", name="ones_f32")
        nc.vector.memset(ones_f32, 1.0)
        ones_row = big.tile([1, P], BF16, tag="ones_row", name="ones_row")
        nc.vector.memset(ones_row, 1.0)

        # ---- persistent tiles -------------------------------------------
        ATL1 = big.tile([P, KC, LOCAL], FP8, tag="ATL1", name="ATL1")
        ATL2 = big.tile([P, KC, LOCAL], FP8, tag="ATL2", name="ATL2")
        invnb1 = big.tile([P, LOCAL], BF16, tag="invnb1", name="invnb1")
        invnb2 = big.tile([P, LOCAL], BF16, tag="invnb2", name="invnb2")
        ivall = big.tile([1, 2 * N], BF16, tag="ivall", name="ivall")
        sel_sb = big.tile([M_CH, 64], FP32, tag="sel_sb", name="sel_sb")

        rsp1 = [
            big.tile([P, N_CH], FP32, tag=f"rsp1_{m}", name=f"rsp1_{m}")
            for m in range(M_CH)
        ]
        rsp2 = [
            big.tile([P, N_CH], FP32, tag=f"rsp2_{m}", name=f"rsp2_{m}")
            for m in range(M_CH)
        ]

        ss1 = big.tile([P, M_CH], FP32, tag="ss1", name="ss1")
        ss2 = big.tile([P, M_CH], FP32, tag="ss2", name="ss2")
        u_ab = big.tile([P, M_CH], FP32, tag="u_ab", name="u_ab")

        # collective buffers
        ag_in = dram.tile([1, 2 * LOCAL], BF16, tag="ag_in", name="ag_in")
        ag_out = dram.tile([1, 2 * N], BF16, tag="ag_out", name="ag_out")
        rs_in = dram.tile([1, 2 * N + 1], FP32, tag="rs_in", name="rs_in")
        rs_out = dram.tile(
            [1, 2 * N + 1], FP32, tag="rs_out", name="rs_out", addr_space="Shared"
        )

        GROUPS = [list(range(NCORES))]

        def sumsq(zt, acc_slice, nm, other=None):
            sq = scr.tile([P, D], BF16, tag="sq", name=f"sq_{nm}", bufs=2)
            nc.vector.scalar_tensor_tensor(
                out=sq, in0=zt, scalar=1.0,
                in1=other if other is not None else zt,
                op0=ALU.mult, op1=ALU.mult, accum_out=acc_slice,
            )

        def rsqrt_newton(ss, w, nm, iters=3):
            ssh = scr.tile([P, w], FP32, tag="rq_ssh", name=f"ssh_{nm}")
            nc.vector.tensor_scalar_mul(ssh, ss, 0.5)
            y = scr.tile([P, w], FP32, tag="rq_y", name=f"y_{nm}")
            nc.vector.tensor_scalar(
                out=y, in0=ssh, scalar1=-(Y0**3), scalar2=1.5 * Y0,
                op0=ALU.mult, op1=ALU.add,
            )
            t = scr.tile([P, w], FP32, tag="rq_t", name=f"t_{nm}")
            u = scr.tile([P, w], FP32, tag="rq_u", name=f"u_{nm}")
            for _ in range(iters - 1):
                nc.vector.tensor_mul(t, y, y)
                nc.vector.tensor_mul(t, t, ssh)
                nc.vector.tensor_mul(u, y, t)
                nc.vector.scalar_tensor_tensor(
                    out=y, in0=y, scalar=1.5, in1=u,
                    op0=ALU.mult, op1=ALU.subtract,
                )
            return y

        # ---- head DMAs (sync queue: local rows, stationary, supers) -----
        r1 = []
        r2 = []
        for t in range(M_CH):
            zt1 = rowz.tile([P, D], FP32, tag="r1", name=f"zl1_{t}")
            nc.sync.dma_start(out=zt1, in_=z1l[P * t : P * (t + 1), :])
            r1.append(zt1)
            zt2 = rowz.tile([P, D], FP32, tag="r2", name=f"zl2_{t}")
            nc.sync.dma_start(out=zt2, in_=z2l[P * t : P * (t + 1), :])
            r2.append(zt2)
        nc.scalar.dma_start(out=sel_sb, in_=selp)

        # ---- local norms -> AllGather (critical chain first) ------------
        # all three consumers of a row tile issue together so the rowz pool
        # (bufs=4) releases slots before later row DMAs need them
        for t in range(M_CH):
            sumsq(r1[t], ss1[:, t : t + 1], f"l1_{t}")
            sumsq(r2[t], ss2[:, t : t + 1], f"l2_{t}")
            sumsq(r1[t], u_ab[:, t : t + 1], f"u_{t}", other=r2[t])
        inv1 = rsqrt_newton(ss1, M_CH, "l1")
        inv2 = rsqrt_newton(ss2, M_CH, "l2")

        ivcl = scr.tile([P, 2 * M_CH], BF16, tag="ivcl", name="ivcl")
        nc.vector.tensor_copy(ivcl[:, 0:M_CH], inv1)
        nc.vector.tensor_copy(ivcl[:, M_CH : 2 * M_CH], inv2)
        # ag_in writes ride the sync HWDGE ring BEFORE the sl/super loads
        # are triggered: descriptor lines interleave round-robin over the
        # shared DMA queues, so trigger order decides completion order —
        # this keeps the collective input from queueing behind megabytes
        # of zT prefetch traffic
        nc.sync.dma_start(
            out=ag_in[:, 0:LOCAL].rearrange("o (t p) -> p (o t)", p=P),
            in_=ivcl[:, 0:M_CH],
        )
        nc.sync.dma_start(
            out=ag_in[:, LOCAL : 2 * LOCAL].rearrange("o (t p) -> p (o t)", p=P),
            in_=ivcl[:, M_CH : 2 * M_CH],
        )
        nc.gpsimd.dma_start(
            out=invnb1, in_=ag_in[:, 0:LOCAL].to_broadcast([P, LOCAL])
        )
        nc.gpsimd.dma_start(
            out=invnb2, in_=ag_in[:, LOCAL : 2 * LOCAL].to_broadcast([P, LOCAL])
        )
        nc.gpsimd.collective_compute(
            "AllGather",
            ALU.bypass,
            replica_groups=GROUPS,
            ins=[ag_in.opt()],
            outs=[ag_out.opt()],
        )
        nc.gpsimd.dma_start(out=ivall, in_=ag_out)

        # big streaming loads trigger only after the ag_in writes
        sl1 = big.tile([P, KC, LOCAL], FP32, tag="sl1", name="sl1")
        nc.sync.dma_start(
            out=sl1, in_=z1lT.rearrange("(k p) n -> p k n", p=P)
        )
        sl2 = big.tile([P, KC, LOCAL], FP32, tag="sl2", name="sl2")
        nc.sync.dma_start(
            out=sl2, in_=z2lT.rearrange("(k p) n -> p k n", p=P)
        )

        st1 = {}
        st2 = {}

        def prefetch(s):
            st1[s] = stg.tile([P, KC, SUPW], FP32, tag="st1", name=f"st1_{s}")
            nc.sync.dma_start(
                out=st1[s],
                in_=z1T.rearrange("(k p) n -> p k n", p=P)[
                    :, :, SUPW * s : SUPW * (s + 1)
                ],
            )
            st2[s] = stg.tile([P, KC, SUPW], FP32, tag="st2", name=f"st2_{s}")
            nc.sync.dma_start(
                out=st2[s],
                in_=z2T.rearrange("(k p) n -> p k n", p=P)[
                    :, :, SUPW * s : SUPW * (s + 1)
                ],
            )

        prefetch(0)
        prefetch(1)

        # dab + stationary fp8 operands (off the AG critical path)
        dab = big.tile([P, M_CH], FP32, tag="dab", name="dab")
        nc.vector.tensor_mul(dab, u_ab, inv1)
        nc.vector.tensor_mul(dab, dab, inv2)

        for k in range(KC):
            nc.vector.scalar_tensor_tensor(
                out=ATL1[:, k, :], in0=sl1[:, k, :], scalar=FSC, in1=invnb1,
                op0=ALU.mult, op1=ALU.mult,
            )
            nc.vector.scalar_tensor_tensor(
                out=ATL2[:, k, :], in0=sl2[:, k, :], scalar=FSC, in1=invnb2,
                op0=ALU.mult, op1=ALU.mult,
            )

        # ---- main loop --------------------------------------------------
        AT1 = {}
        AT2 = {}

        def prep(n):
            """Broadcast 1/norms for chunk n and scale zT slices to fp8."""
            s, h = n // 2, n % 2
            off = 512 * h
            AT1[n] = atp.tile([P, KC, 512], FP8, tag="AT1", name=f"AT1_{n}")
            AT2[n] = atp.tile([P, KC, 512], FP8, tag="AT2", name=f"AT2_{n}")
            # rows 512n..512(n+1) belong to core cblk = n//2, half h; the
            # AllGathered layout per core block is [inv1(1024) | inv2(1024)]
            cblk = n // 2
            base = 2 * LOCAL * cblk
            iv1 = ivall[0:1, base + 512 * h : base + 512 * h + 512]
            iv2 = ivall[0:1, base + LOCAL + 512 * h : base + LOCAL + 512 * h + 512]
            pb1 = pbc.tile([P, 512], FP32, tag="pb", name=f"pb1_{n}")
            nc.tensor.matmul(pb1, ones_row, iv1, start=True, stop=True)
            pbb1 = scr.tile([P, 512], BF16, tag="pbb1", name=f"pbb1_{n}")
            nc.vector.tensor_copy(pbb1, pb1)
            pb2 = pbc.tile([P, 512], FP32, tag="pb", name=f"pb2_{n}")
            nc.tensor.matmul(pb2, ones_row, iv2, start=True, stop=True)
            pbb2 = scr.tile([P, 512], BF16, tag="pbb2", name=f"pbb2_{n}")
            nc.vector.tensor_copy(pbb2, pb2)
            for k in range(KC):
                nc.vector.scalar_tensor_tensor(
                    out=AT1[n][:, k, :], in0=st1[s][:, k, off : off + 512],
                    scalar=FSC, in1=pbb1, op0=ALU.mult, op1=ALU.mult,
                )
            for k in range(KC):
                nc.vector.scalar_tensor_tensor(
                    out=AT2[n][:, k, :], in0=st2[s][:, k, off : off + 512],
                    scalar=FSC, in1=pbb2, op0=ALU.mult, op1=ALU.mult,
                )

        # column sums accumulate on the PE (ones-matmul per m), deferred one
        # m-step so the PE instruction never head-of-line waits on ACT
        pend = []
        colp = {}

        def flush_colsum():
            if not pend:
                return
            n, m, eab = pend.pop(0)
            if m == 0:
                colp[n] = pcol.tile([1, 512], FP32, tag="col", name=f"colp_{n}")
            nc.tensor.matmul(
                colp[n], ones_col, eab[:, 512:1024],
                start=(m == 0), stop=(m == M_CH - 1),
            )
            if m == M_CH - 1:
                csb = scr.tile([1, 512], FP32, tag="csb", name=f"csb_{n}")
                nc.vector.tensor_copy(csb, colp[n])
                nc.sync.dma_start(out=rs_in[:, 512 * n : 512 * (n + 1)], in_=csb)

        def main_chunk(n):
            for m in range(M_CH):
                mm = pmm.tile([P, 1536], FP32, tag="mm", name=f"mm_{n}_{m}")
                lhs1 = ATL1[:, :, P * m : P * (m + 1)]
                lhs2 = ATL2[:, :, P * m : P * (m + 1)]
                for half, (lo, hi) in enumerate(((0, 2), (2, 4))):
                    nc.tensor.matmul(
                        mm[:, 0:512], lhs1[:, lo:hi, :], AT1[n][:, lo:hi, :],
                        start=(half == 0), stop=(half == 1), perf_mode=DR,
                    )
                for half, (lo, hi) in enumerate(((0, 2), (2, 4))):
                    nc.tensor.matmul(
                        mm[:, 512:1024], lhs1[:, lo:hi, :], AT2[n][:, lo:hi, :],
                        start=(half == 0), stop=(half == 1), perf_mode=DR,
                    )
                for half, (lo, hi) in enumerate(((0, 2), (2, 4))):
                    nc.tensor.matmul(
                        mm[:, 1024:1536], lhs2[:, lo:hi, :], AT2[n][:, lo:hi, :],
                        start=(half == 0), stop=(half == 1), perf_mode=DR,
                    )
                flush_colsum()
                eab = eabp.tile([P, 1024], BF16, tag="eab", name=f"eab_{n}_{m}")
                nc.scalar.activation(
                    out=eab, in_=mm[:, 0:1024], func=ACTF.Exp, scale=ES,
                    accum_out=rsp1[m][:, n : n + 1],
                )
                ebb = ebbp.tile([P, 512], BF16, tag="ebb", name=f"ebb_{n}_{m}")
                nc.scalar.activation(
                    out=ebb, in_=mm[:, 1024:1536], func=ACTF.Exp, scale=ES,
                )
                nc.vector.reduce_sum(
                    out=rsp2[m][:, n : n + 1], in_=ebb, axis=mybir.AxisListType.X
                )
                pend.append((n, m, eab))

        # software pipeline: operand prep one chunk ahead, supers two ahead
        prep(0)
        prep(1)
        for n in range(N_CH):
            if n % 2 == 0 and n // 2 + 2 < N_SUP:
                prefetch(n // 2 + 2)
            if n + 2 < N_CH:
                prep(n + 2)
            main_chunk(n)
        while pend:
            flush_colsum()

        # ---- tail -------------------------------------------------------
        rs1 = big.tile([P, M_CH], FP32, tag="rs1", name="rs1")
        rs2 = big.tile([P, M_CH], FP32, tag="rs2", name="rs2")
        for m in range(M_CH):
            nc.vector.reduce_sum(
                out=rs1[:, m : m + 1], in_=rsp1[m], axis=mybir.AxisListType.X
            )
            nc.vector.reduce_sum(
                out=rs2[:, m : m + 1], in_=rsp2[m], axis=mybir.AxisListType.X
            )

        # beta = ln(denom1) - 2 dab / tau ; sum over local rows
        denom1 = scr.tile([P, M_CH], FP32, tag="denom1", name="denom1")
        nc.vector.tensor_scalar_add(denom1, rs1, -EXPD)
        nc.scalar.activation(out=denom1, in_=denom1, func=ACTF.Ln)
        combo = scr.tile([P, M_CH], FP32, tag="combo", name="combo")
        ppart = big.tile([P, 1], FP32, tag="ppart", name="ppart")
        nc.vector.scalar_tensor_tensor(
            out=combo, in0=dab, scalar=-2.0 / TAU, in1=denom1,
            op0=ALU.mult, op1=ALU.add, accum_out=ppart,
        )
        lps = pcol.tile([1, 512], FP32, tag="col", name="lps")
        nc.tensor.matmul(lps[0:1, 0:1], ones_f32, ppart, start=True, stop=True)
        lsb = big.tile([1, 1], FP32, tag="lsb", name="lsb")
        nc.vector.tensor_copy(lsb, lps[0:1, 0:1])
        nc.scalar.dma_start(out=rs_in[:, 2 * N : 2 * N + 1], in_=lsb)

        # alpha = rs2 - EXPD, positioned at global row slot via selector
        alpha = scr.tile([P, M_CH], FP32, tag="alpha", name="alpha")
        nc.vector.tensor_scalar_add(alpha, rs2, -EXPD)
        alr = dram.tile([1, LOCAL], FP32, tag="alr", name="alr")
        nc.scalar.dma_start(
            out=alr.rearrange("o (t p) -> p (o t)", p=P), in_=alpha
        )
        alT = big.tile([M_CH, P], FP32, tag="alT", name="alT")
        nc.scalar.dma_start(
            out=alT, in_=alr.rearrange("o (t p) -> t (o p)", p=P)
        )
        alf = pmm.tile([P, 1536], FP32, tag="mm", name="alf")
        nc.tensor.matmul(alf[0:64, 0:P], sel_sb, alT, start=True, stop=True)
        af_sb = big.tile([64, P], FP32, tag="af_sb", name="af_sb")
        nc.vector.tensor_copy(af_sb, alf[0:64, 0:P])
        nc.scalar.dma_start(
            out=rs_in[:, N : 2 * N].rearrange("o (t p) -> t (o p)", p=P),
            in_=af_sb,
        )

        nc.gpsimd.collective_compute(
            "AllReduce",
            ALU.add,
            replica_groups=GROUPS,
            ins=[rs_in.opt()],
            outs=[rs_out.opt()],
        )

        # final scalar: every core computes it (SPMD); core 0's is read
        cs_t = big.tile([P, 64], FP32, tag="cs_t", name="cs_t")
        nc.scalar.dma_start(
            out=cs_t, in_=rs_out[:, 0:N].rearrange("o (t p) -> p (o t)", p=P)
        )
        al_t = big.tile([P, 64], FP32, tag="al_t", name="al_t")
        nc.scalar.dma_start(
            out=al_t, in_=rs_out[:, N : 2 * N].rearrange("o (t p) -> p (o t)", p=P)
        )
        sb_t = big.tile([1, 1], FP32, tag="sb_t", name="sb_t")
        nc.scalar.dma_start(out=sb_t, in_=rs_out[:, 2 * N : 2 * N + 1])

        dn2 = big.tile([P, 64], FP32, tag="dn2", name="dn2")
        nc.vector.tensor_add(dn2, cs_t, al_t)
        nc.scalar.activation(out=dn2, in_=dn2, func=ACTF.Ln)
        lnp = big.tile([P, 1], FP32, tag="lnp", name="lnp")
        nc.vector.reduce_sum(out=lnp, in_=dn2, axis=mybir.AxisListType.X)
        tl2 = pcol.tile([1, 512], FP32, tag="col", name="tl2")
        nc.tensor.matmul(tl2[0:1, 0:1], ones_f32, lnp, start=True, stop=True)
        tot = big.tile([1, 1], FP32, tag="tot", name="tot")
        nc.vector.tensor_add(tot, tl2[0:1, 0:1], sb_t)
        nc.scalar.mul(tot, tot, 0.5 / N)
        nc.scalar.dma_start(out=loss, in_=tot)

    nc.compile()
    return nc


_NC_CACHE = None


def _get_nc():
    global _NC_CACHE
    if _NC_CACHE is None:
        _NC_CACHE = _build()
    return _NC_CACHE


def _in_maps(z1, z2):
    z1 = np.ascontiguousarray(np.asarray(z1), dtype=np.float32)
    z2 = np.ascontiguousarray(np.asarray(z2), dtype=np.float32)
    z1T = np.ascontiguousarray(z1.T)
    z2T = np.ascontiguousarray(z2.T)
    maps = []
    for c in range(NCORES):
        sl = slice(LOCAL * c, LOCAL * (c + 1))
        sel = np.zeros((M_CH, 64), dtype=np.float32)
        for i in range(M_CH):
            sel[i, M_CH * c + i] = 1.0
        maps.append(
            {
                "z1T": z1T,
                "z2T": z2T,
                "z1l": np.ascontiguousarray(z1[sl]),
                "z2l": np.ascontiguousarray(z2[sl]),
                "z1lT": np.ascontiguousarray(z1T[:, sl]),
                "z2lT": np.ascontiguousarray(z2T[:, sl]),
                "selp": sel,
            }
        )
    return maps


def kernel(z1, z2):
    nc = _get_nc()
    res = run_bass_kernel_spmd(nc, _in_maps(z1, z2), list(range(NCORES)))
    return np.asarray(res.results[0]["loss"], dtype=np.float32).reshape(())


def _install_ntff_hook_shim():
    """The agent image's antenv lacks axon_hooks; recreate the documented
    ctypes hook (same as trn_agent_boot.trn_boot._ntff_profile_via_ctypes)
    so run_bass_kernel_spmd(trace=True) can capture NTFF profiles."""
    import sys, types, ctypes, contextlib

    if "antenv.axon_hooks" in sys.modules:
        return
    so_path = "/opt/axon/libaxon_pjrt.so"
    lib = ctypes.CDLL(so_path)
    if not hasattr(lib, "axon_start_nrt_profile"):
        return
    lib.axon_start_nrt_profile.argtypes = [
        ctypes.POINTER(ctypes.c_int64),
        ctypes.c_size_t,
    ]
    lib.axon_start_nrt_profile.restype = ctypes.c_int64
    lib.axon_stop_nrt_profile.argtypes = [ctypes.c_char_p]
    lib.axon_stop_nrt_profile.restype = ctypes.c_int64

    @contextlib.contextmanager
    def _hook(output_dir, device_ids):
        import jax

        jax.devices()
        if device_ids:
            ids = (ctypes.c_int64 * len(device_ids))(*device_ids)
            rc = lib.axon_start_nrt_profile(ids, len(device_ids))
        else:
            rc = lib.axon_start_nrt_profile(None, 0)
        if rc != 0:
            raise RuntimeError(f"axon_start_nrt_profile rc={rc}")
        try:
            yield
        finally:
            n = lib.axon_stop_nrt_profile(str(output_dir).encode())
            if n < 0:
                raise RuntimeError(f"axon_stop_nrt_profile rc={n}")
            print(f"profile: {n} file(s) written to {output_dir}", file=sys.stderr)

    mod = types.ModuleType("antenv.axon_hooks")
    mod.get_axon_ntff_profile_hook = lambda: _hook
    mod.set_axon_ntff_profile_hook = lambda h: None
    sys.modules["antenv.axon_hooks"] = mod


def kernel_traced(z1, z2):
    """Same as kernel() but with NTFF profiling; returns (loss, exec_time_ns,
    trace_path)."""
    import concourse.bass_utils as bu

    _install_ntff_hook_shim()
    bu.upload_artifacts = lambda tmpdir: "local://" + tmpdir  # no egress
    nc = _get_nc()
    res = run_bass_kernel_spmd(
        nc, _in_maps(z1, z2), list(range(NCORES)), trace=True
    )
    out = np.asarray(res.results[0]["loss"], dtype=np.float32).reshape(())
    trace_path = (
        res.instructions_and_trace[1] if res.instructions_and_trace else None
    )
    return out, res.exec_time_ns, trace_path
